# revision 2
# baseline (speedup 1.0000x reference)
"""EuclideanPairwiseDistances kernel for 8 TRN2 NeuronCores.

Problem: input [B=4, H=256, L=1024, N=128] f32, mask [B, L, N] bool.
  y[b,h,n] = masked mean of input over l=1..1023  -> [B, H, N]
  out[b,p] = sqrt(sum_h (y[b,:,i_p] - y[b,:,j_p])^2 + eps) over tril pairs.

Sharding: core c handles batch b=c//2 and H-half h0=128*(c%2).  Each core
reads its contiguous 64 MiB x-slice, computes masked sums via PE
(partition-dim reduction with a ones vector), then partial squared
pairwise distances over its 128 h-dims.  Host adds the two halves per
batch, applies sqrt, and extracts the tril pairs.

The mask, the 1/denom division, the CLS (l=0) exclusion and a 2^10 scale
(keeps fp16 intermediates in range) are folded into one host-side f32
tensor md[l,n]; on-chip work is z = x*md (DVE) plus matmul reductions.
"""

import numpy as np

import concourse.mybir as mybir
import concourse.tile as tile
from concourse import bacc
from concourse.bass_utils import run_bass_kernel_spmd
from concourse.masks import make_identity

B, H, L, N = 4, 256, 1024, 128
HSH = 128          # h-dims per core
PL = 8             # l-values per partition (L = 128 * PL)
HG = 4             # h-planes per DMA group (2 MiB per dma_start)
EPS = 1e-8
C = 1024.0         # scale folded into md; keeps z=x*md*C/denom ~ O(1) in fp16

HEAD_PLANES = 4    # first h-planes loaded one-at-a-time to start compute early
TAIL_PLANES = 8    # last h-planes loaded one-at-a-time to shorten the tail
X_BUFS = 8
Z_BUFS = 4
SPLIT_DMA = True   # issue each group's load as two half-DMAs on both HWDGE rings

_cached = {}


def _build_bass():
    nc = bacc.Bacc("TRN2", target_bir_lowering=False)

    xs = nc.dram_tensor("xs", [HSH, L, N], mybir.dt.float32, kind="ExternalInput")
    md = nc.dram_tensor("md", [L, N], mybir.dt.float32, kind="ExternalInput")
    dout = nc.dram_tensor("dout", [N, N], mybir.dt.float32, kind="ExternalOutput")

    f16 = mybir.dt.float16
    f32 = mybir.dt.float32

    # group sizes in h-planes: small groups at the head and tail, big in between
    groups = []
    h = 0
    while h < HEAD_PLANES:
        groups.append((h, 1))
        h += 1
    while h < HSH - TAIL_PLANES:
        groups.append((h, HG))
        h += HG
    while h < HSH:
        groups.append((h, 2))
        h += 2

    with tile.TileContext(nc) as tc:
        with (
            tc.tile_pool(name="xp", bufs=X_BUFS) as xp,
            tc.tile_pool(name="zp", bufs=Z_BUFS) as zp,
            tc.tile_pool(name="singles", bufs=1) as singles,
            tc.tile_pool(name="st2", bufs=1) as st2,
            tc.tile_pool(name="psum", bufs=1, space="PSUM") as psum,
        ):
            # --- one-time setup ---
            md_t = singles.tile([128, PL, N], f32)
            # gpsimd ring: keeps the sync/scalar HWDGE rings free for x
            nc.gpsimd.dma_start(
                out=md_t, in_=md.rearrange("(p s) n -> p s n", p=128)
            )

            ones_col = singles.tile([128, 1], f16)
            nc.vector.memset(ones_col, 1.0)
            ones_mat = singles.tile([128, 128], f16)
            nc.vector.memset(ones_mat, 1.0)
            ident = singles.tile([128, 128], f16)
            make_identity(nc, ident)

            # --- stage 1: masked sums S[n, h] (C-scaled) ---
            s_psum = psum.tile([N, HSH], f32)
            d_psum = psum.tile([N, N], f32)

            # stage 2, one h-half at a time: PSUM columns [hlo, hhi) are fully
            # accumulated once those planes' matmul groups retire, so the first
            # half's transpose/Gram work hides under the second half's stream.
            def stage2_half(hi):
                hlo, hhi = hi * (HSH // 2), (hi + 1) * (HSH // 2)
                hw = hhi - hlo
                y_nh = st2.tile([N, HSH // 2], f16, tag=f"y{hi}")
                nc.vector.tensor_copy(y_nh, s_psum[:, hlo:hhi])
                yt_ps = psum.tile([HSH // 2, N], f16, tag=f"ytp{hi}")
                nc.tensor.transpose(yt_ps, y_nh, ident)
                yt = st2.tile([HSH // 2, N], f16, tag=f"yt{hi}")
                nc.vector.tensor_copy(yt, yt_ps)
                ym2 = st2.tile([HSH // 2, N], f16, tag=f"ym{hi}")
                nc.vector.tensor_scalar_mul(ym2, yt_ps, -2.0)
                ysq = st2.tile([HSH // 2, N], f16, tag=f"ys{hi}")
                nc.vector.tensor_mul(ysq, yt, yt)
                first, last = (hi == 0), (hi == 1)
                nc.tensor.matmul(d_psum, yt, ym2, start=first, stop=False)
                nc.tensor.matmul(
                    d_psum, ones_mat[:hw], ysq, start=False, stop=False
                )
                nc.tensor.matmul(
                    d_psum, ysq, ones_mat[:hw], start=False, stop=last
                )

            for gi, (h0, gsz) in enumerate(groups):
                x_t = xp.tile([128, HG, PL, N], f32, tag="x")
                src = xs[h0 : h0 + gsz].rearrange("h (p s) n -> p h s n", p=128)
                if SPLIT_DMA and gsz % 2 == 0:
                    hf = gsz // 2
                    nc.sync.dma_start(out=x_t[:, :hf], in_=src[:, :hf])
                    nc.scalar.dma_start(out=x_t[:, hf:gsz], in_=src[:, hf:])
                else:
                    eng = nc.sync if gi % 2 == 0 else nc.scalar
                    eng.dma_start(out=x_t[:, :gsz], in_=src)

                z_t = zp.tile([128, HG, PL, N], f16, tag="z")
                for hh in range(gsz):
                    # every 4th plane's mask-multiply runs on the otherwise-
                    # idle gpsimd engine so the DVE stays below DMA rate
                    eng = nc.gpsimd if (h0 + hh) % 4 == 3 else nc.vector
                    eng.tensor_mul(z_t[:, hh], x_t[:, hh], md_t)

                for hh in range(gsz):
                    h = h0 + hh
                    for ls in range(PL):
                        nc.tensor.matmul(
                            s_psum[:, h : h + 1],
                            z_t[:, hh, ls, :],
                            ones_col,
                            start=(ls == 0),
                            stop=(ls == PL - 1),
                        )
                if h0 + gsz == HSH // 2:
                    stage2_half(0)

            stage2_half(1)
            d_sb = st2.tile([N, N], f32)
            nc.vector.tensor_copy(d_sb, d_psum)
            nc.sync.dma_start(out=dout[:, :], in_=d_sb)

    nc.compile()
    return nc


def get_bass():
    if "nc" not in _cached:
        _cached["nc"] = _build_bass()
    return _cached["nc"]


def _host_prep(input, mask):
    """Returns per-core in_maps."""
    input = np.ascontiguousarray(np.asarray(input, dtype=np.float32))
    mask = np.asarray(mask)
    denom = mask[:, 1:, :].sum(axis=1)                    # [B, N] ints
    denom = np.maximum(denom, 1).astype(np.float32)
    md = mask.astype(np.float32) * (np.float32(C) / denom[:, None, :])
    md[:, 0, :] = 0.0                                     # CLS position excluded
    md = np.ascontiguousarray(md)

    in_maps = []
    for c in range(8):
        b, half = c // 2, c % 2
        in_maps.append(
            {
                "xs": input[b, half * HSH : (half + 1) * HSH],
                "md": md[b],
            }
        )
    return in_maps


def _host_post(results):
    d = np.stack([r["dout"] for r in results])            # [8, 128, 128]
    dsum = (d[0::2].astype(np.float64) + d[1::2].astype(np.float64)) / (C * C)
    dist = np.sqrt(np.maximum(dsum, 0.0) + EPS).astype(np.float32)  # [4, 128, 128]
    i, j = np.tril_indices(N, -1)
    return np.ascontiguousarray(dist[:, i, j])


def kernel(input, mask, _run_kwargs=None):
    nc = get_bass()
    in_maps = _host_prep(input, mask)
    kwargs = _run_kwargs or {}
    res = run_bass_kernel_spmd(nc, in_maps, core_ids=list(range(8)), **kwargs)
    out = _host_post(res.results)
    if kwargs:
        _cached["last_result"] = res
    return out

